# revision 54
# baseline (speedup 1.0000x reference)
"""Multi-head attention forward on 8 Trainium2 NeuronCores (Bass/Tile).

Problem: B=2, N=2048, D=1024, H=16 heads of dh=64, fp32.

Sharding: tensor-parallel over heads - core c owns heads {2c, 2c+1} and both
batches for projections + attention; on-device AllToAlls re-shard by token so
each core computes the output projection (full Wo) for its 512-token slice
with no reduction.

Layouts: activations travel as [feature, token]; matmul contractions land on
the partition axis. The whole matmul path runs in bfloat16. The two heads'
score matmuls are 64-contraction row-split pairs via tile_position. A ones
column in v_aug makes attn@v's PSUM row 64 accumulate softmax denominators
for free.

The attention phase is PE-streaming-bound, so the schedule targets the PE:
- exp runs 2 chunks ahead of attn@v (lag-2 pipeline, e pool bufs=3) so the
  ~1.1us exp latency never sits in the PE's chunk loop;
- each chunk's exp is column-split across ScalarE (exact) and VectorE
  (Schraudolph fast-exp, int16 bitcast to bf16, ~3% max rel err that the
  softmax ratio mostly cancels), running concurrently on disjoint PSUM
  banks with the side rotated per chunk - both engines outrun the PE
  chunk rate and the sc-buffer release latency halves;
- out-projections sit at end-of-window slots: zero PE cost, but the
  AllToAll receives gain ~8us of slack (collective durations swing
  5-38us run to run on this fabric);
- projection groups are spread across windows (batch-1 K/V ride inside
  window 4 like batch-0 rides window 0); each group is placed only where
  its x-tile is certainly resident - the PE queue is in-order, so an
  emitted matmul waiting on DMA blocks everything behind it;
- host pre-permutes inputs so every tensor loads with one DMA instruction
  (per-tensor dma_start costs ~0.7us of engine issue time), and supplies
  identity/selector constants via DRAM (make_identity needs a ~6us gpsimd
  ucode load);
- collective send/receive DMAs stay per-peer: the DRAM side of each DMA
  must walk contiguously (strided 256B-chunk DRAM access is many times
  slower), and SBUF access patterns cannot permute the partition dim;
- a few dummy matmuls bridge the final collective wait so the tail
  out-proj runs with the PE clock-gate still open; there is no warm-up
  collective - ship(0)'s first-op penalty hides under the ~45us receive
  slack, and a warm op would occupy the serial CC stream right before
  ship(0) under fabric congestion.
"""
from collections import deque
from contextlib import ExitStack

import ml_dtypes
import numpy as np

import concourse.bass as bass
import concourse.tile as tile
from concourse import bacc, mybir
from concourse.bass_utils import run_bass_kernel_spmd

F32 = mybir.dt.float32
F32R = mybir.dt.float32r
BF16 = mybir.dt.bfloat16
I16 = mybir.dt.int16

B, N, D, H, DH = 2, 2048, 1024, 16, 64
W = 8                    # cores
TOK = B * N              # 4096 flattened tokens
KC = D // 128            # contraction chunks for projections (8)
MCB = N // 128           # m-chunks (key chunks) per batch (16)
NW = 8                   # 512-query windows
WTOK = TOK // NW         # 512
NP = 4                   # shipping pairs (1024 tokens each)
AVLAG = 2                # chunks attn@v trails scores by (hides exp latency)

# every chunk's exp is column-split across ScalarE (exact, EXP_SP cols) and
# VectorE (Schraudolph fast-exp: int16 bitcast into bf16), running
# concurrently on disjoint PSUM banks; the split side rotates per chunk so
# each query's softmax mixes exact and approximated keys. This halves the
# sc-buffer release latency vs whole-chunk exp, killing the scores-matmul
# WAR stalls on the 2-deep sc pool.
EXP_SPW = {0: 768, 1: 640, 2: 640, 3: 640, 4: 640, 5: 640, 6: 640, 7: 640}
# ScalarE cols/chunk by window; w0 runs 768 because VectorE carries the
# v_aug copies + bias adds there.
EXPA = 128.0 / float(np.log(2.0))        # 184.6650
EXPB = 127.0 * 128.0 - 6.0 + 0.5         # +0.5 compensates f32->i16 truncation

_CACHE = {}


def build_bass():
    nc = bacc.Bacc("TRN2", target_bir_lowering=False)

    # host pre-permutes everything to [partition, k-chunk, col] so each
    # tensor (or token-tile) loads with ONE dma instruction (engine issue is
    # ~0.7us per dma_start; one InstDMACopy fans out over all 16 SDMA slots)
    xT_d = nc.declare_dram_parameter("xT", [128, KC, TOK], BF16, isOutput=False)
    wq_d = nc.declare_dram_parameter("wq", [128, KC, 128], BF16, isOutput=False)
    wk_d = nc.declare_dram_parameter("wk", [128, KC, 128], BF16, isOutput=False)
    wv_d = nc.declare_dram_parameter("wv", [128, KC, 128], BF16, isOutput=False)
    wo_d = nc.declare_dram_parameter("wo", [128, KC, D], BF16, isOutput=False)
    bqkv_d = nc.declare_dram_parameter("bqkv", [128, 3], F32, isOutput=False)
    # host-supplied constants: identity (PE transpose operand) + normalize
    # selector - avoids make_identity's ~6us gpsimd ucode load in the prologue
    ident_d = nc.declare_dram_parameter("identc", [128, 128], BF16, isOutput=False)
    sel_d = nc.declare_dram_parameter("selc", [128, 128], F32, isOutput=False)
    out_d = nc.declare_dram_parameter("out", [512, D], F32, isOutput=True)

    a2a_in = [nc.dram_tensor(f"a2a_in{p}", [W, 128, 128], BF16) for p in range(NP)]
    a2a_out = [nc.dram_tensor(f"a2a_out{p}", [W, 128, 128], BF16) for p in range(NP)]
    # pair 3 ships per 512-token window so only 128KB rides the tail
    a2aw_in = [nc.dram_tensor(f"a2aw_in{i}", [W, 128, 64], BF16) for i in range(2)]
    a2aw_out = [nc.dram_tensor(f"a2aw_out{i}", [W, 128, 64], BF16) for i in range(2)]

    with tile.TileContext(nc) as tc, ExitStack() as ctx:
        sb = ctx.enter_context(tc.tile_pool(name="sb", bufs=1))
        sbe = ctx.enter_context(tc.tile_pool(name="sbe", bufs=2))
        # PSUM: sc 2x[128,1024] (4 banks) + ha0/ha1 [65,512] (2 banks)
        # + ps2 shared 2x[128,512] (2 banks) = all 8 banks
        ps_sc = ctx.enter_context(tc.tile_pool(name="ps_sc", bufs=2, space="PSUM"))
        ps_ha = ctx.enter_context(tc.tile_pool(name="ps_ha", bufs=1, space="PSUM"))
        ps2 = ctx.enter_context(tc.tile_pool(name="ps2", bufs=2, space="PSUM"))

        # ---------- weights + x DMA issues (ordered for earliest first MM) ----------
        wq = sb.tile([128, KC, 128], BF16, tag="wq")
        wk = sb.tile([128, KC, 128], BF16, tag="wk")
        wv = sb.tile([128, KC, 128], BF16, tag="wv")
        x_sb = sb.tile([128, KC, TOK], BF16, tag="x")
        wo = sb.tile([128, KC, D], BF16, tag="wo")
        bias = sb.tile([128, 3], F32, tag="bias")
        ident = sb.tile([128, 128], BF16, tag="ident")
        sel_ld = sb.tile([128, 128], F32, tag="sel_ld")
        sel = sb.tile([128, 128], F32R, tag="sel")

        # priority order: everything window-0 needs goes first. SDMA engines
        # round-robin across queues at packet granularity, so t0/t1 are each
        # split over two queues to finish ~2x earlier; bulk (batch-1 x, wo)
        # is serialized at the BACK so it can't steal HBM bandwidth early.
        nc.sync.dma_start(bias[:], bqkv_d[:])
        nc.sync.dma_start(x_sb[:, :, 0:256], xT_d[:, :, 0:256])           # t0a
        nc.scalar.dma_start(x_sb[:, :, 256:512], xT_d[:, :, 256:512])     # t0b
        nc.gpsimd.dma_start(wq[:], wq_d[:])
        nc.gpsimd.dma_start(wk[:], wk_d[:])
        nc.gpsimd.dma_start(wv[:], wv_d[:])
        nc.scalar.dma_start(ident[:], ident_d[:])
        nc.scalar.dma_start(sel_ld[:], sel_d[:])
        nc.sync.dma_start(x_sb[:, :, 512:768], xT_d[:, :, 512:768])       # t1a
        nc.scalar.dma_start(x_sb[:, :, 768:1024], xT_d[:, :, 768:1024])   # t1b
        nc.sync.dma_start(x_sb[:, :, 1024:1536], xT_d[:, :, 1024:1536])   # t2
        nc.gpsimd.dma_start(x_sb[:, :, 1536:2048], xT_d[:, :, 1536:2048])  # t3
        # batch 1 - strictly after t0-t3 on sync (two halves so window 4's
        # q-proj can't be held by the full 4MB completing)
        nc.sync.dma_start(x_sb[:, :, 2048:3072], xT_d[:, :, 2048:3072])
        nc.sync.dma_start(x_sb[:, :, 3072:4096], xT_d[:, :, 3072:4096])
        nc.gpsimd.dma_start(wo[:], wo_d[:])  # wo last

        # no warm-up collective: ship(0)'s first-op penalty (~11us) hides
        # under the ~45us receive slack at (5,9), and under a congested
        # fabric the warm op was occupying ~10us of the serial CC stream
        # right before ship(0)

        # ---------- constants ----------
        zeros_f = sb.tile([128, 512], F32, tag="zeros_f")
        nc.vector.memset(zeros_f[:], 0.0)
        nc.vector.tensor_copy(sel[:], sel_ld[:])  # DVE copy rounds to fp32r
        ones_f = sb.tile([128, 1], F32, tag="ones_f")
        nc.vector.memset(ones_f[:], 1.0)

        # ---------- persistent activations ----------
        qT = sb.tile([128, TOK], BF16, tag="qT")
        # combined kT: rows 0:64 head A dims, 64:128 head B dims (no pads -
        # score matmuls are 64-contraction row-split pairs via tile_position)
        kTc = sb.tile([128, TOK], BF16, tag="kTc")
        # head A dims 0:64 + ones col 64; head B dims 65:129 + ones col 129
        v_aug = sb.tile([128, 2 * MCB, 130], BF16, tag="v_aug")
        heads = sb.tile([128, TOK], BF16, tag="heads")
        rcp = sb.tile([128, WTOK], F32R, tag="rcp")
        hT = [sb.tile([128, W, 128], BF16, tag=f"hT{p}", name=f"hT{p}")
              for p in range(NP - 1)]
        # both 64-token tail pairs receive into one tile (cols 0:64 pair 0,
        # 64:128 pair 1) so the tail out-proj runs once at full PE width
        hTwm = sb.tile([128, W, 128], BF16, tag="hTwm", name="hTwm")

        # ones columns of v_aug
        # (memset only supports f32 - fill via dtype-converting copies)
        for gm in range(2 * MCB):
            nc.gpsimd.tensor_copy(v_aug[:, gm, 64:65], ones_f[:, 0:1])
            nc.gpsimd.tensor_copy(v_aug[:, gm, 129:130], ones_f[:, 0:1])
        nc.vector.tensor_copy(rcp[:], zeros_f[:])

        # ---------- projection groups (one 512-token tile each) ----------
        def emit_proj_q(t):
            tsl = bass.ts(t, 512)
            pj = ps2.tile([128, 512], F32, tag="ps2", name="pj")
            for k in range(KC):
                nc.tensor.matmul(pj[:], wq[:, k, :], x_sb[:, k, tsl],
                                 start=(k == 0), stop=(k == KC - 1))
            nc.vector.tensor_scalar_add(qT[:, tsl], pj[:], bias[:, 0:1])

        def emit_proj_k(t):
            tsl = bass.ts(t, 512)
            pj = ps2.tile([128, 512], F32, tag="ps2", name="pj")
            for k in range(KC):
                nc.tensor.matmul(pj[:], wk[:, k, :], x_sb[:, k, tsl],
                                 start=(k == 0), stop=(k == KC - 1))
            nc.vector.tensor_scalar_add(kTc[:, tsl], pj[:], bias[:, 1:2])

        def emit_proj_v(t):
            tsl = bass.ts(t, 512)
            pj = ps2.tile([128, 512], F32, tag="ps2", name="pj")
            for k in range(KC):
                nc.tensor.matmul(pj[:], wv[:, k, :], x_sb[:, k, tsl],
                                 start=(k == 0), stop=(k == KC - 1))
            vt = sbe.tile([128, 512], BF16, tag="vt")
            # bias-add on ScalarE: VectorE carries fast-exp + v_aug copies
            nc.scalar.activation(vt[:], pj[:],
                                 mybir.ActivationFunctionType.Identity,
                                 bias=bias[:, 2:3])
            for i in range(4):
                gm = 4 * t + i
                tp = ps2.tile([128, 128], BF16, tag="ps2", name="tp")
                nc.tensor.transpose(tp[:], vt[:, bass.ts(i, 128)], ident[:])
                nc.vector.tensor_copy(v_aug[:, gm, 0:64], tp[:, 0:64])
                nc.vector.tensor_copy(v_aug[:, gm, 65:129], tp[:, 64:128])

        # ---------- attention pieces ----------
        def emit_scores(b, mc, win_sl):
            msl = bass.ds(2048 * b + 128 * mc, 128)
            sc = ps_sc.tile([128, 1024], F32, tag="sc", name="sc")
            # row-split pair: head A in array rows 0-63, head B in rows
            # 64-127 - disjoint cells, the two matmuls execute concurrently
            nc.tensor.matmul(sc[:, 0:512], kTc[0:64, msl], qT[0:64, win_sl],
                             start=True, stop=True, tile_position=(0, 0))
            nc.tensor.matmul(sc[:, 512:1024], kTc[64:128, msl],
                             qT[64:128, win_sl],
                             start=True, stop=True, tile_position=(64, 0))
            return sc

        def emit_av(pr):
            e, pmc, pgm, pha0, pha1, last = pr
            nc.tensor.matmul(pha0[:], v_aug[:, pgm, 0:65], e[:, 0:512],
                             start=(pmc == 0), stop=last)
            nc.tensor.matmul(pha1[:], v_aug[:, pgm, 65:130], e[:, 512:1024],
                             start=(pmc == 0), stop=last)

        def emit_window_end(pha0, pha1, pw):
            hs0 = sbe.tile([65, 512], F32, tag="hs0", bufs=1)
            hs1 = sbe.tile([128, 512], F32, tag="hs1", bufs=1)
            nc.vector.tensor_copy(hs0[:], pha0[:])
            nc.scalar.activation(hs1[64:128, :], pha1[0:64, :],
                                 mybir.ActivationFunctionType.Copy)
            nc.vector.tensor_copy(rcp[32:33, :], pha0[64:65, :])
            nc.vector.tensor_copy(rcp[96:97, :], pha1[64:65, :])
            return (hs0, hs1, pw)

        def emit_normalize_final(pha0, pha1):
            # last window: normalize straight out of PSUM - no next window
            # will reuse the ha banks, so skip the hs staging copies
            bc = ps2.tile([128, 512], F32, tag="ps2", name="bc")
            nc.tensor.matmul(bc[:], sel[:], rcp[:], start=True, stop=True)
            bc_s = sbe.tile([128, 512], F32, tag="bc_s", bufs=1)
            nc.vector.reciprocal_approx_fast(bc_s[:], bc[:])
            hsl = bass.ds(WTOK * (NW - 1), 512)
            nc.vector.tensor_mul(heads[0:64, hsl], pha0[0:64, :], bc_s[0:64, :])
            nc.vector.tensor_mul(heads[64:128, hsl], pha1[0:64, :], bc_s[64:128, :])

        def emit_normalize(pend):
            hs0, hs1, pw = pend
            bc = ps2.tile([128, 512], F32, tag="ps2", name="bc")
            nc.tensor.matmul(bc[:], sel[:], rcp[:], start=True, stop=True)
            bc_s = sbe.tile([128, 512], F32, tag="bc_s", bufs=1)
            nc.vector.reciprocal_approx_fast(bc_s[:], bc[:])
            hsl = bass.ds(WTOK * pw, 512)
            nc.vector.tensor_mul(heads[0:64, hsl], hs0[0:64, :], bc_s[0:64, :])
            nc.gpsimd.tensor_mul(heads[64:128, hsl], hs1[64:128, :], bc_s[64:128, :])

        def emit_ship(p, tail=False):
            col0 = 1024 * p
            for j in range(W):
                if tail:  # ScalarE is idle once the last exp retires
                    eng = nc.scalar if j % 2 == 0 else nc.gpsimd
                else:
                    eng = nc.sync if j % 2 == 0 else nc.gpsimd
                eng.dma_start(a2a_in[p][j], heads[:, bass.ds(col0 + 128 * j, 128)])
            nc.gpsimd.collective_compute(
                "AllToAll",
                mybir.AluOpType.bypass,
                ins=[a2a_in[p][:]],
                outs=[a2a_out[p][:]],
                replica_groups=[list(range(W))],
            )
            # receives wait ~25us on the collective - keep them off gpsimd
            # so later windows' normalize muls aren't head-of-line blocked
            for j in range(W):
                eng = nc.scalar if tail else nc.sync
                eng.dma_start(hT[p][:, j, :], a2a_out[p][j])

        def emit_outproj(p, tail=False):
            for dc in range(2):
                op = ps2.tile([128, 512], F32, tag="ps2", name="op")
                for k in range(KC):
                    nc.tensor.matmul(op[:], hT[p][:, k, :],
                                     wo[:, k, bass.ts(dc, 512)],
                                     start=(k == 0), stop=(k == KC - 1))
                ot = sbe.tile([128, 512], F32, tag="ot")
                nc.scalar.activation(ot[:], op[:],
                                     mybir.ActivationFunctionType.Copy)
                eng = nc.scalar if tail else nc.sync
                eng.dma_start(out_d[bass.ts(p, 128), bass.ts(dc, 512)], ot[:])

        def emit_ship_w(i):
            # window 6+i of pair 3: 64 tokens per peer
            col0 = 512 * (6 + i)
            for j in range(W):
                eng = nc.scalar if j % 2 == 0 else nc.gpsimd
                eng.dma_start(a2aw_in[i][j], heads[:, bass.ds(col0 + 64 * j, 64)])
            nc.gpsimd.collective_compute(
                "AllToAll",
                mybir.AluOpType.bypass,
                ins=[a2aw_in[i][:]],
                outs=[a2aw_out[i][:]],
                replica_groups=[list(range(W))],
            )
            for j in range(W):
                eng = nc.sync if j % 2 == 0 else nc.scalar
                eng.dma_start(hTwm[:, j, bass.ts(i, 64)], a2aw_out[i][j])

        def emit_outproj_w(i):
            # 64-token tail window out-proj (i=0 -> rows 384:448, 1 -> 448:512)
            for dc in range(2):
                op = ps2.tile([128, 512], F32, tag="ps2", name="op")
                for k in range(KC):
                    nc.tensor.matmul(op[0:64, :], hTwm[:, k, bass.ts(i, 64)],
                                     wo[:, k, bass.ts(dc, 512)],
                                     start=(k == 0), stop=(k == KC - 1))
                ot = sbe.tile([128, 512], F32, tag="ot")
                nc.scalar.activation(ot[0:64, :], op[0:64, :],
                                     mybir.ActivationFunctionType.Copy)
                nc.scalar.dma_start(
                    out_d[bass.ds(384 + 64 * i, 64), bass.ts(dc, 512)],
                    ot[0:64, :])

        # ---------- schedule ----------
        # projection interleave, spread so every window carries a similar PE
        # load. Constraints: Q(t) before window t; batch-0 K(j)/V(j) before
        # window 0's chunk 4j (ride inside w0); batch-1 K/V ride inside w4
        # the same way (all of them before w5 chunk 0); out-proj pairs land
        # in the lightest windows (5,6,7).
        # window-0 slots are data-deadline-placed: the PE queue is in-order,
        # so a group emitted at slot s barriers every later chunk behind its
        # x-tile DMA; each group therefore goes at the LATEST slot before its
        # first consumer (K(j) before chunk 4j, V(j) before chunk 4j+2's
        # attn@v), and V0 (t0-resident, never a barrier) fills slot 1.
        interleave = {
            (0, 1): lambda: emit_proj_v(0), (0, 3): lambda: emit_proj_k(1),
            (0, 4): lambda: emit_proj_v(1), (0, 7): lambda: emit_proj_k(2),
            (0, 9): lambda: emit_proj_v(2), (0, 10): lambda: emit_proj_q(1),
            (0, 11): lambda: emit_proj_k(3), (0, 12): lambda: emit_proj_v(3),
            (1, 1): lambda: emit_proj_q(2), (1, 9): lambda: emit_proj_q(3),
            (2, 1): lambda: emit_proj_q(4), (2, 5): lambda: emit_proj_k(4),
            (2, 9): lambda: emit_proj_v(4),
            (3, 1): lambda: emit_proj_q(5), (3, 5): lambda: emit_proj_k(5),
            (3, 9): lambda: emit_proj_v(5),
            (4, 1): lambda: emit_proj_k(6), (4, 5): lambda: emit_proj_v(6),
            (4, 9): lambda: emit_proj_k(7), (4, 11): lambda: emit_proj_v(7),
            (5, 5): lambda: emit_proj_q(6),
            (6, 5): lambda: emit_proj_q(7),
        }
        outproj_at = {(5, 9): 0, (6, 9): 1}

        # prologue projections: PE is otherwise idle while x streams in
        # (V0 rides slot (0,1) instead - attn@v only needs it at chunk 2)
        emit_proj_q(0)
        emit_proj_k(0)

        pend = deque()   # (e, mc, gm, ha0, ha1, last) awaiting attn@v
        pending = None   # window-end extraction awaiting normalize
        endw = None      # (ha0, ha1, w) of window just finished

        for w in range(NW):
            b = w // 4
            win_sl = bass.ds(WTOK * w, 512)
            ha0 = ps_ha.tile([65, 512], F32, tag="ha0", name="ha0")
            ha1 = ps_ha.tile([65, 512], F32, tag="ha1", name="ha1")
            sp = EXP_SPW[w]
            # chunks are batched in pairs: both chunks' score row-split
            # pairs go back-to-back (consecutive pairs use disjoint 64-row
            # groups, so the second pair's LDWEIGHTS pull ahead under the
            # first's in-flight matmuls), then both pending attn@v pairs.
            # This halves the exposed LDW boundaries per chunk and gives
            # attn@v one extra chunk of exp slack.
            for mc0 in range(0, MCB, 2):
                mcs = (mc0, mc0 + 1)
                scl = [emit_scores(b, mc, win_sl) for mc in mcs]
                if len(pend) >= 2:
                    for _ in range(2):
                        pr = pend.popleft()
                        emit_av(pr)
                        if pr[5]:
                            pending = emit_window_end(pr[3], pr[4], w - 1)
                # proj filler after attn@v: soaks PE slack without delaying
                # the exp-feeding scores or the window-closing attn@v chain
                for mc in mcs:
                    fn = interleave.get((w, mc))
                    if fn is not None:
                        fn()
                for sc, mc in zip(scl, mcs):
                    e = sbe.tile([128, 1024], BF16, tag="e", bufs=AVLAG + 2)
                    if mc % 2 == 0:
                        slo, shi = 0, sp
                    else:
                        slo, shi = 1024 - sp, 1024
                    nc.scalar.activation(e[:, slo:shi], sc[:, slo:shi],
                                         mybir.ActivationFunctionType.Exp)
                    if slo > 0:
                        nc.vector.tensor_scalar(
                            e[:, 0:slo].bitcast(I16), sc[:, 0:slo], EXPA,
                            EXPB, mybir.AluOpType.mult, mybir.AluOpType.add)
                    if shi < 1024:
                        nc.vector.tensor_scalar(
                            e[:, shi:1024].bitcast(I16), sc[:, shi:1024],
                            EXPA, EXPB,
                            mybir.AluOpType.mult, mybir.AluOpType.add)
                    pend.append((e, mc, MCB * b + mc, ha0, ha1,
                                 mc == MCB - 1))
                for mc in mcs:
                    if mc == 5 and pending is not None:
                        emit_normalize(pending)
                        pw = pending[2]
                        pending = None
                        if pw == 6:
                            emit_ship_w(0)
                        elif pw % 2 == 1 and pw < 6:
                            emit_ship(pw // 2)
                    p = outproj_at.get((w, mc))
                    if p is not None:
                        emit_outproj(p)

        # epilogue: drain the attn@v pipeline, final normalize; window 7's
        # 128KB collective is the only one left exposed
        while pend:
            emit_av(pend.popleft())
        pha0, pha1 = ha0, ha1
        nc.vector.tensor_copy(rcp[32:33, :], pha0[64:65, :])
        nc.vector.tensor_copy(rcp[96:97, :], pha1[64:65, :])
        emit_normalize_final(pha0, pha1)
        emit_ship_w(1)
        # bridge the ~12us final send->receive latency with REAL work:
        # outproj(2) (receive landed at window-7 start) and pair-0's tail
        # out-proj (a window ago) run here instead of as window filler,
        # plus a short dummy stretch, so outproj_w(1) hits the warm clock
        emit_outproj(2)
        emit_outproj_w(0)
        warmp = ps2.tile([128, 512], F32, tag="ps2", name="warmp")
        for _ in range(14):
            nc.tensor.matmul(warmp[:, 0:256], ident[:], qT[:, 0:256],
                             start=True, stop=True)
        emit_outproj_w(1)

    nc.compile()
    return nc


def _bf16(a):
    return np.ascontiguousarray(a).astype(ml_dtypes.bfloat16)


def _pkc(a):
    # [D, cols] -> [128, KC, cols]: row k*128+p lands at [p, k, :]
    return np.ascontiguousarray(
        a.reshape(KC, 128, a.shape[1]).transpose(1, 0, 2)
    ).astype(ml_dtypes.bfloat16)


_IDENT = np.eye(128, dtype=ml_dtypes.bfloat16)
_SEL = np.zeros((128, 128), dtype=np.float32)
_SEL[32, 0:64] = 1.0
_SEL[96, 64:128] = 1.0


def _prep_inputs(x, Wq, bq, Wk, bk, Wv, bv, Wo, bo):
    xT = _pkc(x.reshape(TOK, D).T)
    wo = _pkc(Wo)
    in_maps = []
    for c in range(W):
        sl = slice(128 * c, 128 * (c + 1))
        bqkv = np.stack([bq[sl] / 8.0, bk[sl], bv[sl]], axis=1).astype(np.float32)
        in_maps.append({
            "xT": xT,
            "wq": _pkc(Wq[:, sl] / 8.0),
            "wk": _pkc(Wk[:, sl]),
            "wv": _pkc(Wv[:, sl]),
            "wo": wo,
            "bqkv": np.ascontiguousarray(bqkv),
            "identc": _IDENT,
            "selc": _SEL,
        })
    return in_maps


def run(x, Wq, bq, Wk, bk, Wv, bv, Wo, bo, **run_kwargs):
    if "nc" not in _CACHE:
        _CACHE["nc"] = build_bass()
    nc = _CACHE["nc"]
    in_maps = _prep_inputs(x, Wq, bq, Wk, bk, Wv, bv, Wo, bo)
    res = run_bass_kernel_spmd(nc, in_maps, list(range(W)), **run_kwargs)
    out = np.empty((TOK, D), np.float32)
    for c in range(W):
        r = res.results[c]["out"]
        for p in range(3):
            out[1024 * p + 128 * c:1024 * p + 128 * (c + 1)] = r[128 * p:128 * (p + 1)]
        out[3072 + 64 * c:3072 + 64 * (c + 1)] = r[384:448]
        out[3584 + 64 * c:3584 + 64 * (c + 1)] = r[448:512]
    out = out.reshape(B, N, D) + bo.astype(np.float32)
    return out.astype(np.float32), res


def kernel(x, Wq, bq, Wk, bk, Wv, bv, Wo, bo):
    x, Wq, bq, Wk, bk, Wv, bv, Wo, bo = (
        np.asarray(a, dtype=np.float32)
        for a in (x, Wq, bq, Wk, bk, Wv, bv, Wo, bo)
    )
    out, _ = run(x, Wq, bq, Wk, bk, Wv, bv, Wo, bo)
    return out


# revision 55
# speedup vs baseline: 1.0425x; 1.0425x over previous
"""Multi-head attention forward on 8 Trainium2 NeuronCores (Bass/Tile).

Problem: B=2, N=2048, D=1024, H=16 heads of dh=64, fp32.

Sharding: tensor-parallel over heads - core c owns heads {2c, 2c+1} and both
batches for projections + attention; on-device AllToAlls re-shard by token so
each core computes the output projection (full Wo) for its 512-token slice
with no reduction.

Layouts: activations travel as [feature, token]; matmul contractions land on
the partition axis. The whole matmul path runs in bfloat16. The two heads'
score matmuls are 64-contraction row-split pairs via tile_position. A ones
column in v_aug makes attn@v's PSUM row 64 accumulate softmax denominators
for free.

The attention phase is PE-streaming-bound, so the schedule targets the PE:
- exp runs 2 chunks ahead of attn@v (lag-2 pipeline, e pool bufs=3) so the
  ~1.1us exp latency never sits in the PE's chunk loop;
- each chunk's exp is column-split across ScalarE (exact) and VectorE
  (Schraudolph fast-exp, int16 bitcast to bf16, ~3% max rel err that the
  softmax ratio mostly cancels), running concurrently on disjoint PSUM
  banks with the side rotated per chunk - both engines outrun the PE
  chunk rate and the sc-buffer release latency halves;
- out-projections sit at end-of-window slots: zero PE cost, but the
  AllToAll receives gain ~8us of slack (collective durations swing
  5-38us run to run on this fabric);
- projection groups are spread across windows (batch-1 K/V ride inside
  window 4 like batch-0 rides window 0); each group is placed only where
  its x-tile is certainly resident - the PE queue is in-order, so an
  emitted matmul waiting on DMA blocks everything behind it;
- host pre-permutes inputs so every tensor loads with one DMA instruction
  (per-tensor dma_start costs ~0.7us of engine issue time), and supplies
  identity/selector constants via DRAM (make_identity needs a ~6us gpsimd
  ucode load);
- collective send/receive DMAs stay per-peer: the DRAM side of each DMA
  must walk contiguously (strided 256B-chunk DRAM access is many times
  slower), and SBUF access patterns cannot permute the partition dim;
- a few dummy matmuls bridge the final collective wait so the tail
  out-proj runs with the PE clock-gate still open; there is no warm-up
  collective - ship(0)'s first-op penalty hides under the ~45us receive
  slack, and a warm op would occupy the serial CC stream right before
  ship(0) under fabric congestion.
"""
from collections import deque
from contextlib import ExitStack

import ml_dtypes
import numpy as np

import concourse.bass as bass
import concourse.tile as tile
from concourse import bacc, mybir
from concourse.bass_utils import run_bass_kernel_spmd

F32 = mybir.dt.float32
F32R = mybir.dt.float32r
BF16 = mybir.dt.bfloat16
I16 = mybir.dt.int16

B, N, D, H, DH = 2, 2048, 1024, 16, 64
W = 8                    # cores
TOK = B * N              # 4096 flattened tokens
KC = D // 128            # contraction chunks for projections (8)
MCB = N // 128           # m-chunks (key chunks) per batch (16)
NW = 8                   # 512-query windows
WTOK = TOK // NW         # 512
NP = 4                   # shipping pairs (1024 tokens each)
AVLAG = 2                # chunks attn@v trails scores by (hides exp latency)

# every chunk's exp is column-split across ScalarE (exact, EXP_SP cols) and
# VectorE (Schraudolph fast-exp: int16 bitcast into bf16), running
# concurrently on disjoint PSUM banks; the split side rotates per chunk so
# each query's softmax mixes exact and approximated keys. This halves the
# sc-buffer release latency vs whole-chunk exp, killing the scores-matmul
# WAR stalls on the 2-deep sc pool.
EXP_SPW = {0: 768, 1: 512, 2: 512, 3: 640, 4: 640, 5: 640, 6: 640, 7: 640}
# ScalarE cols/chunk by window; w0 runs 768 because VectorE carries the
# v_aug copies + bias adds there.
EXPA = 128.0 / float(np.log(2.0))        # 184.6650
EXPB = 127.0 * 128.0 - 6.0 + 0.5         # +0.5 compensates f32->i16 truncation

_CACHE = {}


def build_bass():
    nc = bacc.Bacc("TRN2", target_bir_lowering=False)

    # host pre-permutes everything to [partition, k-chunk, col] so each
    # tensor (or token-tile) loads with ONE dma instruction (engine issue is
    # ~0.7us per dma_start; one InstDMACopy fans out over all 16 SDMA slots)
    xT_d = nc.declare_dram_parameter("xT", [128, KC, TOK], BF16, isOutput=False)
    wq_d = nc.declare_dram_parameter("wq", [128, KC, 128], BF16, isOutput=False)
    wk_d = nc.declare_dram_parameter("wk", [128, KC, 128], BF16, isOutput=False)
    wv_d = nc.declare_dram_parameter("wv", [128, KC, 128], BF16, isOutput=False)
    wo_d = nc.declare_dram_parameter("wo", [128, KC, D], BF16, isOutput=False)
    bqkv_d = nc.declare_dram_parameter("bqkv", [128, 3], F32, isOutput=False)
    # host-supplied constants: identity (PE transpose operand) + normalize
    # selector - avoids make_identity's ~6us gpsimd ucode load in the prologue
    ident_d = nc.declare_dram_parameter("identc", [128, 128], BF16, isOutput=False)
    sel_d = nc.declare_dram_parameter("selc", [128, 128], F32, isOutput=False)
    out_d = nc.declare_dram_parameter("out", [512, D], F32, isOutput=True)

    a2a_in = [nc.dram_tensor(f"a2a_in{p}", [W, 128, 128], BF16) for p in range(NP)]
    a2a_out = [nc.dram_tensor(f"a2a_out{p}", [W, 128, 128], BF16) for p in range(NP)]
    # pair 3 ships per 512-token window so only 128KB rides the tail
    a2aw_in = [nc.dram_tensor(f"a2aw_in{i}", [W, 128, 64], BF16) for i in range(2)]
    a2aw_out = [nc.dram_tensor(f"a2aw_out{i}", [W, 128, 64], BF16) for i in range(2)]

    with tile.TileContext(nc) as tc, ExitStack() as ctx:
        sb = ctx.enter_context(tc.tile_pool(name="sb", bufs=1))
        sbe = ctx.enter_context(tc.tile_pool(name="sbe", bufs=2))
        # PSUM: sc 2x[128,1024] (4 banks) + ha0/ha1 [65,512] (2 banks)
        # + ps2 shared 2x[128,512] (2 banks) = all 8 banks
        ps_sc = ctx.enter_context(tc.tile_pool(name="ps_sc", bufs=2, space="PSUM"))
        ps_ha = ctx.enter_context(tc.tile_pool(name="ps_ha", bufs=1, space="PSUM"))
        ps2 = ctx.enter_context(tc.tile_pool(name="ps2", bufs=2, space="PSUM"))

        # ---------- weights + x DMA issues (ordered for earliest first MM) ----------
        wq = sb.tile([128, KC, 128], BF16, tag="wq")
        wk = sb.tile([128, KC, 128], BF16, tag="wk")
        wv = sb.tile([128, KC, 128], BF16, tag="wv")
        x_sb = sb.tile([128, KC, TOK], BF16, tag="x")
        wo = sb.tile([128, KC, D], BF16, tag="wo")
        bias = sb.tile([128, 3], F32, tag="bias")
        ident = sb.tile([128, 128], BF16, tag="ident")
        sel_ld = sb.tile([128, 128], F32, tag="sel_ld")
        sel = sb.tile([128, 128], F32R, tag="sel")

        # priority order: everything window-0 needs goes first. SDMA engines
        # round-robin across queues at packet granularity, so t0/t1 are each
        # split over two queues to finish ~2x earlier; bulk (batch-1 x, wo)
        # is serialized at the BACK so it can't steal HBM bandwidth early.
        nc.sync.dma_start(bias[:], bqkv_d[:])
        nc.sync.dma_start(x_sb[:, :, 0:256], xT_d[:, :, 0:256])           # t0a
        nc.scalar.dma_start(x_sb[:, :, 256:512], xT_d[:, :, 256:512])     # t0b
        nc.gpsimd.dma_start(wq[:], wq_d[:])
        nc.gpsimd.dma_start(wk[:], wk_d[:])
        nc.gpsimd.dma_start(wv[:], wv_d[:])
        nc.scalar.dma_start(ident[:], ident_d[:])
        nc.scalar.dma_start(sel_ld[:], sel_d[:])
        nc.sync.dma_start(x_sb[:, :, 512:768], xT_d[:, :, 512:768])       # t1a
        nc.scalar.dma_start(x_sb[:, :, 768:1024], xT_d[:, :, 768:1024])   # t1b
        nc.sync.dma_start(x_sb[:, :, 1024:1536], xT_d[:, :, 1024:1536])   # t2
        nc.gpsimd.dma_start(x_sb[:, :, 1536:2048], xT_d[:, :, 1536:2048])  # t3
        # batch 1 - strictly after t0-t3 on sync (two halves so window 4's
        # q-proj can't be held by the full 4MB completing)
        nc.sync.dma_start(x_sb[:, :, 2048:3072], xT_d[:, :, 2048:3072])
        nc.sync.dma_start(x_sb[:, :, 3072:4096], xT_d[:, :, 3072:4096])
        nc.gpsimd.dma_start(wo[:], wo_d[:])  # wo last

        # no warm-up collective: ship(0)'s first-op penalty (~11us) hides
        # under the ~45us receive slack at (5,9), and under a congested
        # fabric the warm op was occupying ~10us of the serial CC stream
        # right before ship(0)

        # ---------- constants ----------
        zeros_f = sb.tile([128, 512], F32, tag="zeros_f")
        nc.vector.memset(zeros_f[:], 0.0)
        nc.vector.tensor_copy(sel[:], sel_ld[:])  # DVE copy rounds to fp32r
        ones_f = sb.tile([128, 1], F32, tag="ones_f")
        nc.vector.memset(ones_f[:], 1.0)

        # ---------- persistent activations ----------
        qT = sb.tile([128, TOK], BF16, tag="qT")
        # combined kT: rows 0:64 head A dims, 64:128 head B dims (no pads -
        # score matmuls are 64-contraction row-split pairs via tile_position)
        kTc = sb.tile([128, TOK], BF16, tag="kTc")
        # head A dims 0:64 + ones col 64; head B dims 65:129 + ones col 129
        v_aug = sb.tile([128, 2 * MCB, 130], BF16, tag="v_aug")
        heads = sb.tile([128, TOK], BF16, tag="heads")
        rcp = sb.tile([128, WTOK], F32R, tag="rcp")
        hT = [sb.tile([128, W, 128], BF16, tag=f"hT{p}", name=f"hT{p}")
              for p in range(NP - 1)]
        # both 64-token tail pairs receive into one tile (cols 0:64 pair 0,
        # 64:128 pair 1) so the tail out-proj runs once at full PE width
        hTwm = sb.tile([128, W, 128], BF16, tag="hTwm", name="hTwm")

        # ones columns of v_aug
        # (memset only supports f32 - fill via dtype-converting copies)
        for gm in range(2 * MCB):
            nc.gpsimd.tensor_copy(v_aug[:, gm, 64:65], ones_f[:, 0:1])
            nc.gpsimd.tensor_copy(v_aug[:, gm, 129:130], ones_f[:, 0:1])
        nc.vector.tensor_copy(rcp[:], zeros_f[:])

        # ---------- projection groups (one 512-token tile each) ----------
        def emit_proj_q(t):
            tsl = bass.ts(t, 512)
            pj = ps2.tile([128, 512], F32, tag="ps2", name="pj")
            for k in range(KC):
                nc.tensor.matmul(pj[:], wq[:, k, :], x_sb[:, k, tsl],
                                 start=(k == 0), stop=(k == KC - 1))
            nc.vector.tensor_scalar_add(qT[:, tsl], pj[:], bias[:, 0:1])

        def emit_proj_k(t):
            tsl = bass.ts(t, 512)
            pj = ps2.tile([128, 512], F32, tag="ps2", name="pj")
            for k in range(KC):
                nc.tensor.matmul(pj[:], wk[:, k, :], x_sb[:, k, tsl],
                                 start=(k == 0), stop=(k == KC - 1))
            nc.vector.tensor_scalar_add(kTc[:, tsl], pj[:], bias[:, 1:2])

        def emit_proj_v(t):
            tsl = bass.ts(t, 512)
            pj = ps2.tile([128, 512], F32, tag="ps2", name="pj")
            for k in range(KC):
                nc.tensor.matmul(pj[:], wv[:, k, :], x_sb[:, k, tsl],
                                 start=(k == 0), stop=(k == KC - 1))
            vt = sbe.tile([128, 512], BF16, tag="vt")
            # bias-add on ScalarE: VectorE carries fast-exp + v_aug copies
            nc.scalar.activation(vt[:], pj[:],
                                 mybir.ActivationFunctionType.Identity,
                                 bias=bias[:, 2:3])
            for i in range(4):
                gm = 4 * t + i
                tp = ps2.tile([128, 128], BF16, tag="ps2", name="tp")
                nc.tensor.transpose(tp[:], vt[:, bass.ts(i, 128)], ident[:])
                nc.vector.tensor_copy(v_aug[:, gm, 0:64], tp[:, 0:64])
                nc.vector.tensor_copy(v_aug[:, gm, 65:129], tp[:, 64:128])

        # ---------- attention pieces ----------
        def emit_scores(b, mc, win_sl):
            msl = bass.ds(2048 * b + 128 * mc, 128)
            sc = ps_sc.tile([128, 1024], F32, tag="sc", name="sc")
            # row-split pair: head A in array rows 0-63, head B in rows
            # 64-127 - disjoint cells, the two matmuls execute concurrently
            nc.tensor.matmul(sc[:, 0:512], kTc[0:64, msl], qT[0:64, win_sl],
                             start=True, stop=True, tile_position=(0, 0))
            nc.tensor.matmul(sc[:, 512:1024], kTc[64:128, msl],
                             qT[64:128, win_sl],
                             start=True, stop=True, tile_position=(64, 0))
            return sc

        def emit_av(pr):
            e, pmc, pgm, pha0, pha1, last = pr
            nc.tensor.matmul(pha0[:], v_aug[:, pgm, 0:65], e[:, 0:512],
                             start=(pmc == 0), stop=last)
            nc.tensor.matmul(pha1[:], v_aug[:, pgm, 65:130], e[:, 512:1024],
                             start=(pmc == 0), stop=last)

        def emit_window_end(pha0, pha1, pw):
            hs0 = sbe.tile([65, 512], F32, tag="hs0", bufs=1)
            hs1 = sbe.tile([128, 512], F32, tag="hs1", bufs=1)
            nc.vector.tensor_copy(hs0[:], pha0[:])
            nc.scalar.activation(hs1[64:128, :], pha1[0:64, :],
                                 mybir.ActivationFunctionType.Copy)
            nc.vector.tensor_copy(rcp[32:33, :], pha0[64:65, :])
            nc.vector.tensor_copy(rcp[96:97, :], pha1[64:65, :])
            return (hs0, hs1, pw)

        def emit_normalize_final(pha0, pha1):
            # last window: normalize straight out of PSUM - no next window
            # will reuse the ha banks, so skip the hs staging copies
            bc = ps2.tile([128, 512], F32, tag="ps2", name="bc")
            nc.tensor.matmul(bc[:], sel[:], rcp[:], start=True, stop=True)
            bc_s = sbe.tile([128, 512], F32, tag="bc_s", bufs=1)
            nc.vector.reciprocal_approx_fast(bc_s[:], bc[:])
            hsl = bass.ds(WTOK * (NW - 1), 512)
            nc.vector.tensor_mul(heads[0:64, hsl], pha0[0:64, :], bc_s[0:64, :])
            nc.vector.tensor_mul(heads[64:128, hsl], pha1[0:64, :], bc_s[64:128, :])

        def emit_normalize(pend):
            hs0, hs1, pw = pend
            bc = ps2.tile([128, 512], F32, tag="ps2", name="bc")
            nc.tensor.matmul(bc[:], sel[:], rcp[:], start=True, stop=True)
            bc_s = sbe.tile([128, 512], F32, tag="bc_s", bufs=1)
            nc.vector.reciprocal_approx_fast(bc_s[:], bc[:])
            hsl = bass.ds(WTOK * pw, 512)
            nc.vector.tensor_mul(heads[0:64, hsl], hs0[0:64, :], bc_s[0:64, :])
            nc.gpsimd.tensor_mul(heads[64:128, hsl], hs1[64:128, :], bc_s[64:128, :])

        def emit_ship(p, tail=False):
            col0 = 1024 * p
            for j in range(W):
                if tail:  # ScalarE is idle once the last exp retires
                    eng = nc.scalar if j % 2 == 0 else nc.gpsimd
                else:
                    eng = nc.sync if j % 2 == 0 else nc.gpsimd
                eng.dma_start(a2a_in[p][j], heads[:, bass.ds(col0 + 128 * j, 128)])
            nc.gpsimd.collective_compute(
                "AllToAll",
                mybir.AluOpType.bypass,
                ins=[a2a_in[p][:]],
                outs=[a2a_out[p][:]],
                replica_groups=[list(range(W))],
            )
            # receives wait ~25us on the collective - keep them off gpsimd
            # so later windows' normalize muls aren't head-of-line blocked
            for j in range(W):
                eng = nc.scalar if tail else nc.sync
                eng.dma_start(hT[p][:, j, :], a2a_out[p][j])

        def emit_outproj(p, tail=False):
            for dc in range(2):
                op = ps2.tile([128, 512], F32, tag="ps2", name="op")
                for k in range(KC):
                    nc.tensor.matmul(op[:], hT[p][:, k, :],
                                     wo[:, k, bass.ts(dc, 512)],
                                     start=(k == 0), stop=(k == KC - 1))
                ot = sbe.tile([128, 512], F32, tag="ot")
                nc.scalar.activation(ot[:], op[:],
                                     mybir.ActivationFunctionType.Copy)
                eng = nc.scalar if tail else nc.sync
                eng.dma_start(out_d[bass.ts(p, 128), bass.ts(dc, 512)], ot[:])

        def emit_ship_w(i):
            # window 6+i of pair 3: 64 tokens per peer
            col0 = 512 * (6 + i)
            for j in range(W):
                eng = nc.scalar if j % 2 == 0 else nc.gpsimd
                eng.dma_start(a2aw_in[i][j], heads[:, bass.ds(col0 + 64 * j, 64)])
            nc.gpsimd.collective_compute(
                "AllToAll",
                mybir.AluOpType.bypass,
                ins=[a2aw_in[i][:]],
                outs=[a2aw_out[i][:]],
                replica_groups=[list(range(W))],
            )
            for j in range(W):
                eng = nc.sync if j % 2 == 0 else nc.scalar
                eng.dma_start(hTwm[:, j, bass.ts(i, 64)], a2aw_out[i][j])

        def emit_outproj_w(i):
            # 64-token tail window out-proj (i=0 -> rows 384:448, 1 -> 448:512)
            for dc in range(2):
                op = ps2.tile([128, 512], F32, tag="ps2", name="op")
                for k in range(KC):
                    nc.tensor.matmul(op[0:64, :], hTwm[:, k, bass.ts(i, 64)],
                                     wo[:, k, bass.ts(dc, 512)],
                                     start=(k == 0), stop=(k == KC - 1))
                ot = sbe.tile([128, 512], F32, tag="ot")
                nc.scalar.activation(ot[0:64, :], op[0:64, :],
                                     mybir.ActivationFunctionType.Copy)
                nc.scalar.dma_start(
                    out_d[bass.ds(384 + 64 * i, 64), bass.ts(dc, 512)],
                    ot[0:64, :])

        # ---------- schedule ----------
        # projection interleave, spread so every window carries a similar PE
        # load. Constraints: Q(t) before window t; batch-0 K(j)/V(j) before
        # window 0's chunk 4j (ride inside w0); batch-1 K/V ride inside w4
        # the same way (all of them before w5 chunk 0); out-proj pairs land
        # in the lightest windows (5,6,7).
        # window-0 slots are data-deadline-placed: the PE queue is in-order,
        # so a group emitted at slot s barriers every later chunk behind its
        # x-tile DMA; each group therefore goes at the LATEST slot before its
        # first consumer (K(j) before chunk 4j, V(j) before chunk 4j+2's
        # attn@v), and V0 (t0-resident, never a barrier) fills slot 1.
        interleave = {
            (0, 1): lambda: emit_proj_v(0), (0, 3): lambda: emit_proj_k(1),
            (0, 4): lambda: emit_proj_v(1), (0, 7): lambda: emit_proj_k(2),
            (0, 9): lambda: emit_proj_v(2), (0, 10): lambda: emit_proj_q(1),
            (0, 11): lambda: emit_proj_k(3), (0, 12): lambda: emit_proj_v(3),
            (1, 1): lambda: emit_proj_q(2), (1, 9): lambda: emit_proj_q(3),
            (2, 1): lambda: emit_proj_q(4), (2, 5): lambda: emit_proj_k(4),
            (2, 9): lambda: emit_proj_v(4),
            (3, 1): lambda: emit_proj_q(5), (3, 5): lambda: emit_proj_k(5),
            (3, 9): lambda: emit_proj_v(5),
            (4, 1): lambda: emit_proj_k(6), (4, 5): lambda: emit_proj_v(6),
            (4, 9): lambda: emit_proj_k(7), (4, 11): lambda: emit_proj_v(7),
            (5, 5): lambda: emit_proj_q(6),
            (6, 5): lambda: emit_proj_q(7),
        }
        outproj_at = {(5, 9): 0, (6, 9): 1}

        # prologue projections: PE is otherwise idle while x streams in
        # (V0 rides slot (0,1) instead - attn@v only needs it at chunk 2)
        emit_proj_q(0)
        emit_proj_k(0)

        pend = deque()   # (e, mc, gm, ha0, ha1, last) awaiting attn@v
        pending = None   # window-end extraction awaiting normalize
        endw = None      # (ha0, ha1, w) of window just finished

        for w in range(NW):
            b = w // 4
            win_sl = bass.ds(WTOK * w, 512)
            ha0 = ps_ha.tile([65, 512], F32, tag="ha0", name="ha0")
            ha1 = ps_ha.tile([65, 512], F32, tag="ha1", name="ha1")
            sp = EXP_SPW[w]
            # chunks are batched in pairs: both chunks' score row-split
            # pairs go back-to-back (consecutive pairs use disjoint 64-row
            # groups, so the second pair's LDWEIGHTS pull ahead under the
            # first's in-flight matmuls), then both pending attn@v pairs.
            # This halves the exposed LDW boundaries per chunk and gives
            # attn@v one extra chunk of exp slack.
            for mc0 in range(0, MCB, 2):
                mcs = (mc0, mc0 + 1)
                scl = [emit_scores(b, mc, win_sl) for mc in mcs]
                if len(pend) >= 2:
                    for _ in range(2):
                        pr = pend.popleft()
                        emit_av(pr)
                        if pr[5]:
                            pending = emit_window_end(pr[3], pr[4], w - 1)
                # proj filler after attn@v: soaks PE slack without delaying
                # the exp-feeding scores or the window-closing attn@v chain
                for mc in mcs:
                    fn = interleave.get((w, mc))
                    if fn is not None:
                        fn()
                for sc, mc in zip(scl, mcs):
                    e = sbe.tile([128, 1024], BF16, tag="e", bufs=AVLAG + 2)
                    if mc % 2 == 0:
                        slo, shi = 0, sp
                    else:
                        slo, shi = 1024 - sp, 1024
                    nc.scalar.activation(e[:, slo:shi], sc[:, slo:shi],
                                         mybir.ActivationFunctionType.Exp)
                    if slo > 0:
                        nc.vector.tensor_scalar(
                            e[:, 0:slo].bitcast(I16), sc[:, 0:slo], EXPA,
                            EXPB, mybir.AluOpType.mult, mybir.AluOpType.add)
                    if shi < 1024:
                        nc.vector.tensor_scalar(
                            e[:, shi:1024].bitcast(I16), sc[:, shi:1024],
                            EXPA, EXPB,
                            mybir.AluOpType.mult, mybir.AluOpType.add)
                    pend.append((e, mc, MCB * b + mc, ha0, ha1,
                                 mc == MCB - 1))
                for mc in mcs:
                    if mc == 5 and pending is not None:
                        emit_normalize(pending)
                        pw = pending[2]
                        pending = None
                        if pw == 6:
                            emit_ship_w(0)
                        elif pw % 2 == 1 and pw < 6:
                            emit_ship(pw // 2)
                    p = outproj_at.get((w, mc))
                    if p is not None:
                        emit_outproj(p)

        # epilogue: drain the attn@v pipeline, final normalize; window 7's
        # 128KB collective is the only one left exposed
        while pend:
            emit_av(pend.popleft())
        pha0, pha1 = ha0, ha1
        nc.vector.tensor_copy(rcp[32:33, :], pha0[64:65, :])
        nc.vector.tensor_copy(rcp[96:97, :], pha1[64:65, :])
        emit_normalize_final(pha0, pha1)
        emit_ship_w(1)
        # bridge the ~12us final send->receive latency with REAL work:
        # outproj(2) (receive landed at window-7 start) and pair-0's tail
        # out-proj (a window ago) run here instead of as window filler,
        # plus a short dummy stretch, so outproj_w(1) hits the warm clock
        emit_outproj(2)
        emit_outproj_w(0)
        warmp = ps2.tile([128, 512], F32, tag="ps2", name="warmp")
        for _ in range(14):
            nc.tensor.matmul(warmp[:, 0:256], ident[:], qT[:, 0:256],
                             start=True, stop=True)
        emit_outproj_w(1)

    nc.compile()
    return nc


def _bf16(a):
    return np.ascontiguousarray(a).astype(ml_dtypes.bfloat16)


def _pkc(a):
    # [D, cols] -> [128, KC, cols]: row k*128+p lands at [p, k, :]
    return np.ascontiguousarray(
        a.reshape(KC, 128, a.shape[1]).transpose(1, 0, 2)
    ).astype(ml_dtypes.bfloat16)


_IDENT = np.eye(128, dtype=ml_dtypes.bfloat16)
_SEL = np.zeros((128, 128), dtype=np.float32)
_SEL[32, 0:64] = 1.0
_SEL[96, 64:128] = 1.0


def _prep_inputs(x, Wq, bq, Wk, bk, Wv, bv, Wo, bo):
    xT = _pkc(x.reshape(TOK, D).T)
    wo = _pkc(Wo)
    in_maps = []
    for c in range(W):
        sl = slice(128 * c, 128 * (c + 1))
        bqkv = np.stack([bq[sl] / 8.0, bk[sl], bv[sl]], axis=1).astype(np.float32)
        in_maps.append({
            "xT": xT,
            "wq": _pkc(Wq[:, sl] / 8.0),
            "wk": _pkc(Wk[:, sl]),
            "wv": _pkc(Wv[:, sl]),
            "wo": wo,
            "bqkv": np.ascontiguousarray(bqkv),
            "identc": _IDENT,
            "selc": _SEL,
        })
    return in_maps


def run(x, Wq, bq, Wk, bk, Wv, bv, Wo, bo, **run_kwargs):
    if "nc" not in _CACHE:
        _CACHE["nc"] = build_bass()
    nc = _CACHE["nc"]
    in_maps = _prep_inputs(x, Wq, bq, Wk, bk, Wv, bv, Wo, bo)
    res = run_bass_kernel_spmd(nc, in_maps, list(range(W)), **run_kwargs)
    out = np.empty((TOK, D), np.float32)
    for c in range(W):
        r = res.results[c]["out"]
        for p in range(3):
            out[1024 * p + 128 * c:1024 * p + 128 * (c + 1)] = r[128 * p:128 * (p + 1)]
        out[3072 + 64 * c:3072 + 64 * (c + 1)] = r[384:448]
        out[3584 + 64 * c:3584 + 64 * (c + 1)] = r[448:512]
    out = out.reshape(B, N, D) + bo.astype(np.float32)
    return out.astype(np.float32), res


def kernel(x, Wq, bq, Wk, bk, Wv, bv, Wo, bo):
    x, Wq, bq, Wk, bk, Wv, bv, Wo, bo = (
        np.asarray(a, dtype=np.float32)
        for a in (x, Wq, bq, Wk, bk, Wv, bv, Wo, bo)
    )
    out, _ = run(x, Wq, bq, Wk, bk, Wv, bv, Wo, bo)
    return out
